# revision 2
# baseline (speedup 1.0000x reference)
"""Trainium2 Bass kernel v5 for the temporal-gradient-matching loss.

reference:
    dx = pred[:, 1:] - pred[:, :-1]   (frame diffs, B x (N-1) x HW)
    dy = y[:, 1:]    - y[:, :-1]
    loss = sum | |dx| - |dy| | / (B * (N-1))

Measured-rate-driven design. On this TRN2 build (per-op in-program
slope measurements): DVE tensor_add ~1.5 us / tensor_scalar(bitwise)
~1.6 us per [128,5456] fp16 pass, but tensor_sub ~5.6 us (no packed
uop). ACT Abs ~3.5 us, GPSIMD add ~9 us, DMA ~400 GB/s. So the kernel
uses NO subtracts:

  * Host negates ODD frames of both inputs: x'_n = (-1)^n x_n. Then
    x'_{n+1} + x'_n = +-(x_{n+1} - x_n), and abs() kills the sign.
  * -|dy| is tensor_scalar(bitwise_or 0x8000) on fp16 (force sign bit).
  * t = |dx| + (-|dy|);  |t| via ACT Abs;  acc += |t| (add).

Per window (N = (frames-1)*S elems/partition):
  J1  DVE  add: dx~ = x'1 + x'0          (+-dx, shifted views, ~1.5us)
  J2  add: dy~ split DVE [0:c) / GPSIMD [c:N)
  A1  ACT  Abs: p = |dx~|  -> d2 left    (~3.5us)
  A2  DVE  bitor: q' = -|dy~| -> d2 right (~1.6us)
  J4  DVE  add: t = p + q' (same-tensor operands -> packed mode, ~1.5us)
  A3  ACT  Abs: |t| in place             (~3.5us)
  ACC DVE  add: acc += |t| (w=0: copy)   (~1.5us)
Tail: one tensor_reduce(acc) -> [128,1] f32, DMA out, host sums.
Engine busy/window ~ DVE 6.6, ACT 7.0, GP ~5.6 vs DMA ~7.2+ -> DMA-bound.

fp16 on device (host casts; tolerance 2e-2, fp16 error cancels to ~1e-7
in the 33M-term sum). 17.3 MB HBM/core. Host layout per core:
[NWIN, 128, 2*N*S] fp16, partitions = (batch 4) x (pixel-chunk 32),
free = x block then y block -> one contiguous HWDGE DMA per window.
Pixel sharding -> no halo; zero pads contribute 0.
"""

import contextlib

import numpy as np

import concourse.bass as bass
import concourse.mybir as mybir
from concourse.bass_utils import run_bass_kernel_spmd

# ---- problem geometry (hardcoded; kernel.py must be self-contained) ----
BB = 4            # batch
NN = 32           # frames
HH = 518
WW = 518
HWP = HH * WW     # 268324 pixels per frame
NCORES = 8

# ---- kernel tiling ----
S = 176           # pixels per chunk (even: keeps fp16 DVE packing aligned)
J = 32            # chunks per batch per window -> 4*32 = 128 partitions
NWIN = 6          # windows per core
PK = S * J * NWIN           # 33792 pixels per core
PTOT = PK * NCORES          # 270336 >= HWP, zero padded (pads contribute 0)

NP = 128
FREE = NN * S               # free elems per partition per input tile
DFREE = (NN - 1) * S        # free elems per partition per diff tile
NBUF = 2                    # xy SBUF slots
CSPLIT = 1984               # dy elems [0:CSPLIT) on DVE, rest on GPSIMD (even)


def build_nc(bb=BB, nn=NN, s=S, j=J, nwin=NWIN, nbuf=NBUF, csplit=CSPLIT,
             reps=1, timing=False):
    """Build the per-core Bass program (SPMD: all cores run this).

    reps > 1 re-runs the whole compute that many times reading the same
    one-window input (timing=True) - wall-delta steady-state timing only.
    """
    np_parts = bb * j
    free = nn * s
    dfree = (nn - 1) * s
    assert 0 <= csplit <= dfree and csplit % 2 == 0
    f16 = mybir.dt.float16
    f32 = mybir.dt.float32
    u16 = mybir.dt.uint16
    AT = mybir.AluOpType
    AF = mybir.ActivationFunctionType
    nw = nwin * reps
    nin = 1 if timing else nwin
    c = csplit

    nc = bass.Bass()
    xyd = nc.dram_tensor("xy", [nin, np_parts, 2 * free], f16,
                         kind="ExternalInput")
    od = nc.dram_tensor("partials", [np_parts, 1], f32, kind="ExternalOutput")

    with contextlib.ExitStack() as ctx:
        xy = [
            ctx.enter_context(nc.sbuf_tensor(f"xy{i}", [np_parts, 2 * free], f16))
            for i in range(nbuf)
        ]
        dx = [
            ctx.enter_context(nc.sbuf_tensor(f"dx{i}", [np_parts, dfree], f16))
            for i in range(2)
        ]
        dy = ctx.enter_context(nc.sbuf_tensor("dy", [np_parts, dfree], f16))
        # d2 = [ p | q' ]  so J4's operands share one tensor (packed mode)
        d2 = [
            ctx.enter_context(nc.sbuf_tensor(f"d2{i}", [np_parts, 2 * dfree], f16))
            for i in range(2)
        ]
        t2 = [
            ctx.enter_context(nc.sbuf_tensor(f"t2{i}", [np_parts, dfree], f16))
            for i in range(2)
        ]
        acc = ctx.enter_context(nc.sbuf_tensor("acc", [np_parts, dfree], f16))
        scr = ctx.enter_context(nc.sbuf_tensor("scr", [np_parts, 2], f16))
        accf = ctx.enter_context(nc.sbuf_tensor("accf", [np_parts, 1], f32))

        xysem = [ctx.enter_context(nc.semaphore(f"xysem{i}")) for i in range(nbuf)]
        vsem = ctx.enter_context(nc.semaphore("vsem"))   # DVE: 5 incs/iter
        asem = ctx.enter_context(nc.semaphore("asem"))   # ACT: 2 incs/iter
        psem = ctx.enter_context(nc.semaphore("psem"))   # GP : 1 inc/window
        osem = ctx.enter_context(nc.semaphore("osem"))

        dyu = dy[:].bitcast(u16)
        d2u = [d.bitcast(u16) for d in (d2[0][:], d2[1][:])]

        block = ctx.enter_context(nc.Block())

        # Software-pipelined schedule (skewed stages, no mid-window
        # cross-engine stalls). DVE iteration i emits exactly 5 vsem incs:
        #   J1(i), J2a(i), A2(i)          [valid for i < nw]
        #   J4(i-1)                       [valid for 1 <= i <= nw]
        #   ACC(i-2)                      [valid for 2 <= i <= nw+1]
        # ACT iteration k emits exactly 2 asem incs:
        #   A1(k) [k < nw], A3(k-1) [1 <= k <= nw]
        # Positions: J1(i)=5i+1, J2a=5i+2, A2=5i+3, J4(w)=5w+9, ACC(w)=5w+15
        #            A1(k)=2k+1, A3(w)=2w+4, J2b(w)=w+1

        @block.sync
        def _(sync):
            for w in range(nw):
                if w >= nbuf:
                    sync.wait_ge(vsem, 5 * (w - nbuf) + 2)   # J1,J2a read xy
                    sync.wait_ge(psem, (w - nbuf) + 1)       # J2b read xy
                sync.dma_start(out=xy[w % nbuf][:], in_=xyd[w % nin]).then_inc(
                    xysem[w % nbuf], 16
                )
            sync.wait_ge(vsem, 5 * (nw + 2) + 1)   # final reduce done
            sync.dma_start(out=od[:], in_=accf[:]).then_inc(osem, 16)
            sync.wait_ge(osem, 16)

        @block.vector
        def _(vector):
            def vnop():
                nc.vector.engine_nop().then_inc(vsem, 1)

            for i in range(nw + 2):
                # --- J1(i), J2a(i), A2(i) ---
                if i < nw:
                    sl = i % nbuf
                    x = xy[sl]
                    vector.wait_ge(xysem[sl], 16 * (i // nbuf + 1))
                    if i >= 2:
                        vector.wait_ge(asem, 2 * (i - 2) + 1)  # A1(i-2): dx free
                    nc.vector.tensor_add(                      # J1: +-dx
                        dx[i % 2][:], x[:, s:free], x[:, 0:dfree]
                    ).then_inc(vsem, 1)
                    if c > 0:
                        nc.vector.tensor_add(                  # J2a
                            dy[:, 0:c],
                            x[:, free + s : free + s + c],
                            x[:, free : free + c],
                        ).then_inc(vsem, 1)
                    else:
                        vnop()
                    vector.wait_ge(psem, i + 1)                # J2b(i) done
                    nc.vector.tensor_scalar(                   # A2: q' = -|dy|
                        d2u[i % 2][:, dfree : 2 * dfree], dyu, 0x8000, None,
                        AT.bitwise_or,
                    ).then_inc(vsem, 1)
                else:
                    vnop(); vnop(); vnop()
                # --- J4(i-1): t = p + q' ---
                w = i - 1
                if 0 <= w < nw:
                    vector.wait_ge(asem, 2 * w + 1)            # A1(w) done
                    nc.vector.tensor_add(
                        t2[w % 2][:], d2[w % 2][:, 0:dfree],
                        d2[w % 2][:, dfree : 2 * dfree],
                    ).then_inc(vsem, 1)
                else:
                    vnop()
                # --- ACC(i-2) ---
                w = i - 2
                if 0 <= w < nw:
                    vector.wait_ge(asem, 2 * w + 4)            # A3(w) done
                    if w % nwin == 0:
                        nc.vector.tensor_copy(acc[:], t2[w % 2][:]).then_inc(
                            vsem, 1
                        )
                    else:
                        nc.vector.tensor_add(
                            acc[:], acc[:], t2[w % 2][:]
                        ).then_inc(vsem, 1)
                else:
                    vnop()
            nc.vector.tensor_reduce(
                accf[:], acc[:], mybir.AxisListType.XYZW,
                AT.add, apply_absolute_value=True,
            ).then_inc(vsem, 1)

        @block.scalar
        def _(scalar):
            for k in range(nw + 1):
                if k < nw:
                    scalar.wait_ge(vsem, 5 * k + 1)            # J1(k) done
                    nc.scalar.activation(                      # A1: p = |dx|
                        d2[k % 2][:, 0:dfree], dx[k % 2][:], AF.Abs
                    ).then_inc(asem, 1)
                else:
                    nc.scalar.activation(scr[:], scr[:], AF.Abs).then_inc(asem, 1)
                w = k - 1
                if 0 <= w < nw:
                    scalar.wait_ge(vsem, 5 * w + 9)            # J4(w) done
                    nc.scalar.activation(                      # A3: |t|
                        t2[w % 2][:], t2[w % 2][:], AF.Abs
                    ).then_inc(asem, 1)
                else:
                    nc.scalar.activation(scr[:], scr[:], AF.Abs).then_inc(asem, 1)

        @block.gpsimd
        def _(gpsimd):
            for w in range(nw):
                sl = w % nbuf
                x = xy[sl]
                gpsimd.wait_ge(xysem[sl], 16 * (w // nbuf + 1))
                if w >= 1:
                    gpsimd.wait_ge(vsem, 5 * (w - 1) + 3)     # A2(w-1) read dy
                if c < dfree:
                    nc.gpsimd.tensor_add(                      # J2b
                        dy[:, c:dfree],
                        x[:, free + s + c : free + s + dfree],
                        x[:, free + c : free + dfree],
                    ).then_inc(psem, 1)
                else:
                    nc.gpsimd.engine_nop().then_inc(psem, 1)

    return nc


_NC = None


def _get_nc():
    global _NC
    if _NC is None:
        _NC = build_nc()
    return _NC


def shard_host(flat_padded, k, bb=BB, nn=NN, s=S, j=J, nwin=NWIN, pk=PK):
    """[B, N, PTOT] -> core k's [NWIN, B*J, N*S] transposed shard."""
    sl = flat_padded[:, :, k * pk : (k + 1) * pk]          # [B, N, PK]
    v = sl.reshape(bb, nn, nwin, j, s)                     # [B, N, W, J, S]
    v = v.transpose(2, 0, 3, 1, 4)                         # [W, B, J, N, S]
    return np.ascontiguousarray(v).reshape(nwin, bb * j, nn * s)


def _prep_shards(pred, y):
    """Full fp32 inputs -> per-core [NWIN, 128, 2*N*S] fp16 shards with
    odd frames negated (turns frame-diff subtracts into adds)."""
    xf = np.asarray(pred, dtype=np.float32).reshape(BB, NN, HWP)
    yf = np.asarray(y, dtype=np.float32).reshape(BB, NN, HWP)
    sgn = np.where(np.arange(NN) % 2 == 1, -1.0, 1.0).astype(np.float32)
    xf = (xf * sgn[None, :, None]).astype(np.float16)
    yf = (yf * sgn[None, :, None]).astype(np.float16)
    xpad = np.zeros((BB, NN, PTOT), dtype=np.float16)
    ypad = np.zeros((BB, NN, PTOT), dtype=np.float16)
    xpad[:, :, :HWP] = xf
    ypad[:, :, :HWP] = yf
    out = []
    for k in range(NCORES):
        xv = shard_host(xpad, k)
        yv = shard_host(ypad, k)
        out.append({"xy": np.concatenate([xv, yv], axis=2)})
    return out


def _combine(results):
    """Per-core [128, 1] partials -> scalar loss."""
    total = 0.0
    for r in results:
        total += np.asarray(r["partials"], dtype=np.float64).sum()
    return np.array(total / (BB * (NN - 1)), dtype=np.float32)


def run(pred, y, trace=False):
    """Returns (loss, exec_time_ns or None)."""
    nc = _get_nc()
    in_maps = _prep_shards(pred, y)
    res = run_bass_kernel_spmd(
        nc, in_maps, core_ids=list(range(NCORES)), trace=trace
    )
    return _combine(res.results), res.exec_time_ns


def kernel(pred, y):
    out, _ = run(pred, y, trace=False)
    return out


# revision 3
# speedup vs baseline: 1.1294x; 1.1294x over previous
"""Trainium2 Bass kernel v5 for the temporal-gradient-matching loss.

reference:
    dx = pred[:, 1:] - pred[:, :-1]   (frame diffs, B x (N-1) x HW)
    dy = y[:, 1:]    - y[:, :-1]
    loss = sum | |dx| - |dy| | / (B * (N-1))

Measured-rate-driven design. On this TRN2 build (per-op in-program
slope measurements): DVE tensor_add ~1.5 us / tensor_scalar(bitwise)
~1.6 us per [128,5456] fp16 pass, but tensor_sub ~5.6 us (no packed
uop). ACT Abs ~3.5 us, GPSIMD add ~9 us, DMA ~400 GB/s. So the kernel
uses NO subtracts:

  * Host negates ODD frames of both inputs: x'_n = (-1)^n x_n. Then
    x'_{n+1} + x'_n = +-(x_{n+1} - x_n), and abs() kills the sign.
  * -|dy| is tensor_scalar(bitwise_or 0x8000) on fp16 (force sign bit).
  * t = |dx| + (-|dy|);  |t| via ACT Abs;  acc += |t| (add).

Per window (N = (frames-1)*S elems/partition):
  J1  DVE  add: dx~ = x'1 + x'0          (+-dx, shifted views, ~1.5us)
  J2  add: dy~ split DVE [0:c) / GPSIMD [c:N)
  A1  ACT  Abs: p = |dx~|  -> d2 left    (~3.5us)
  A2  DVE  bitor: q' = -|dy~| -> d2 right (~1.6us)
  J4  DVE  add: t = p + q' (same-tensor operands -> packed mode, ~1.5us)
  A3  ACT  Abs: |t| in place             (~3.5us)
  ACC DVE  add: acc += |t| (w=0: copy)   (~1.5us)
Tail: one tensor_reduce(acc) -> [128,1] f32, DMA out, host sums.
Engine busy/window ~ DVE 6.6, ACT 7.0, GP ~5.6 vs DMA ~7.2+ -> DMA-bound.

fp16 on device (host casts; tolerance 2e-2, fp16 error cancels to ~1e-7
in the 33M-term sum). 17.3 MB HBM/core. Host layout per core:
[NWIN, 128, 2*N*S] fp16, partitions = (batch 4) x (pixel-chunk 32),
free = x block then y block -> one contiguous HWDGE DMA per window.
Pixel sharding -> no halo; zero pads contribute 0.
"""

import contextlib

import numpy as np

import concourse.bass as bass
import concourse.mybir as mybir
from concourse.bass_utils import run_bass_kernel_spmd

# ---- problem geometry (hardcoded; kernel.py must be self-contained) ----
BB = 4            # batch
NN = 32           # frames
HH = 518
WW = 518
HWP = HH * WW     # 268324 pixels per frame
NCORES = 8

# ---- kernel tiling ----
S = 176           # pixels per chunk (even: keeps fp16 DVE packing aligned)
J = 32            # chunks per batch per window -> 4*32 = 128 partitions
NWIN = 6          # windows per core
PK = S * J * NWIN           # 33792 pixels per core
PTOT = PK * NCORES          # 270336 >= HWP, zero padded (pads contribute 0)

NP = 128
FREE = NN * S               # free elems per partition per input tile
DFREE = (NN - 1) * S        # free elems per partition per diff tile
NBUF = 2                    # xy SBUF slots
CSPLIT = 1984               # dy elems [0:CSPLIT) on DVE, rest on GPSIMD (even)


def build_nc(bb=BB, nn=NN, s=S, j=J, nwin=NWIN, nbuf=NBUF, csplit=CSPLIT,
             reps=1, timing=False):
    """Build the per-core Bass program (SPMD: all cores run this).

    reps > 1 re-runs the whole compute that many times reading the same
    one-window input (timing=True) - wall-delta steady-state timing only.
    """
    np_parts = bb * j
    free = nn * s
    dfree = (nn - 1) * s
    assert 0 <= csplit <= dfree and csplit % 2 == 0
    f16 = mybir.dt.float16
    f32 = mybir.dt.float32
    u16 = mybir.dt.uint16
    AT = mybir.AluOpType
    AF = mybir.ActivationFunctionType
    nw = nwin * reps
    nin = 1 if timing else nwin
    c = csplit

    nc = bass.Bass()
    xyd = nc.dram_tensor("xy", [nin, np_parts, 2 * free], f16,
                         kind="ExternalInput")
    od = nc.dram_tensor("partials", [np_parts, nwin], f32, kind="ExternalOutput")

    with contextlib.ExitStack() as ctx:
        xy = [
            ctx.enter_context(nc.sbuf_tensor(f"xy{i}", [np_parts, 2 * free], f16))
            for i in range(nbuf)
        ]
        dx = [
            ctx.enter_context(nc.sbuf_tensor(f"dx{i}", [np_parts, dfree], f16))
            for i in range(2)
        ]
        dy = [
            ctx.enter_context(nc.sbuf_tensor(f"dy{i}", [np_parts, dfree], f16))
            for i in range(2)
        ]
        # d2 = [ p | q' ]  so J4's operands share one tensor (packed mode)
        d2 = [
            ctx.enter_context(nc.sbuf_tensor(f"d2{i}", [np_parts, 2 * dfree], f16))
            for i in range(2)
        ]
        t2 = [
            ctx.enter_context(nc.sbuf_tensor(f"t2{i}", [np_parts, dfree], f16))
            for i in range(2)
        ]
        acc = ctx.enter_context(nc.sbuf_tensor("acc", [np_parts, nwin], f32))
        scr = ctx.enter_context(nc.sbuf_tensor("scr", [np_parts, 2], f16))

        xysem = [ctx.enter_context(nc.semaphore(f"xysem{i}")) for i in range(nbuf)]
        vsem = ctx.enter_context(nc.semaphore("vsem"))   # DVE: 5 incs/iter
        asem = ctx.enter_context(nc.semaphore("asem"))   # ACT: 2 incs/iter
        psem = ctx.enter_context(nc.semaphore("psem"))   # GP : 1 inc/window
        osem = ctx.enter_context(nc.semaphore("osem"))

        dyu = [d[:].bitcast(u16) for d in dy]
        d2u = [d.bitcast(u16) for d in (d2[0][:], d2[1][:])]

        block = ctx.enter_context(nc.Block())

        # Deep software pipeline: every cross-engine wait targets an event
        # >= 1 full iteration old, and each engine does ONE coalesced wait
        # per iteration (stall-free in steady state).
        # DVE iteration i emits exactly 5 vsem incs:
        #   J1(i)=5i+1, J2a(i)=5i+2   [i < nw]
        #   A2(i-1)=5i+3              [0 <= i-1 < nw]   q' = -|dy(i-1)|
        #   J4(i-2)=5i+4              [0 <= i-2 < nw]   t = p + q'
        #   ACC(i-3)=5i+5             [0 <= i-3 < nw]
        # ACT iteration k emits exactly 2 asem incs:
        #   A1(k)=2k+1 [k < nw], A3(k-2)=2k+2 [0 <= k-2 < nw]
        # GP: J2b(w)=w+1.
        # Single waits: DVE@i: asem >= 2i (covers A1(i-2)=2i-3 dx-free,
        #   A3(i-3)=2(i-3)+6=2i ACC src) + psem >= i (J2b(i-1)) + xysem.
        # ACT@k: vsem >= 5k+4 (covers J1(k)=5k+1 and J4(k-2)=5k+4).
        # GP@w: vsem >= 5(w-1)+3 (A2(w-2) freed dy slot) + xysem.

        @block.sync
        def _(sync):
            for w in range(nw):
                if w >= nbuf:
                    sync.wait_ge(vsem, 4 * (w - nbuf) + 2)   # J1,J2a read xy
                    sync.wait_ge(psem, (w - nbuf) + 1)       # J2b read xy
                sync.dma_start(out=xy[w % nbuf][:], in_=xyd[w % nin]).then_inc(
                    xysem[w % nbuf], 16
                )
            sync.wait_ge(asem, 2 * (nw + 1) + 2)   # last A3 accum done
            sync.dma_start(out=od[:], in_=acc[:]).then_inc(osem, 16)
            sync.wait_ge(osem, 16)

        @block.vector
        def _(vector):
            def vnop():
                nc.vector.engine_nop().then_inc(vsem, 1)

            for i in range(nw + 2):
                if i >= 2:
                    vector.wait_ge(asem, 2 * i - 2)    # one coalesced wait
                if 1 <= i <= nw:
                    vector.wait_ge(psem, i)            # J2b(i-1) done
                # --- J1(i), J2a(i) ---
                if i < nw:
                    sl = i % nbuf
                    x = xy[sl]
                    vector.wait_ge(xysem[sl], 16 * (i // nbuf + 1))
                    nc.vector.tensor_add(                      # J1: +-dx
                        dx[i % 2][:], x[:, s:free], x[:, 0:dfree]
                    ).then_inc(vsem, 1)
                    if c > 0:
                        nc.vector.tensor_add(                  # J2a
                            dy[i % 2][:, 0:c],
                            x[:, free + s : free + s + c],
                            x[:, free : free + c],
                        ).then_inc(vsem, 1)
                    else:
                        vnop()
                else:
                    vnop(); vnop()
                # --- A2(i-1): q' = -|dy| ---
                w = i - 1
                if 0 <= w < nw:
                    nc.vector.tensor_scalar(
                        d2u[w % 2][:, dfree : 2 * dfree], dyu[w % 2], 0x8000,
                        None, AT.bitwise_or,
                    ).then_inc(vsem, 1)
                else:
                    vnop()
                # --- J4(i-2): t = p + q' ---
                w = i - 2
                if 0 <= w < nw:
                    nc.vector.tensor_add(
                        t2[w % 2][:], d2[w % 2][:, 0:dfree],
                        d2[w % 2][:, dfree : 2 * dfree],
                    ).then_inc(vsem, 1)
                else:
                    vnop()

        @block.scalar
        def _(scalar):
            for k in range(nw + 2):
                scalar.wait_ge(vsem, 4 * k + 4)        # one coalesced wait
                if k < nw:
                    nc.scalar.activation(              # A1: p = |dx|
                        d2[k % 2][:, 0:dfree], dx[k % 2][:], AF.Abs
                    ).then_inc(asem, 1)
                else:
                    nc.scalar.activation(scr[:], scr[:], AF.Abs).then_inc(asem, 1)
                w = k - 2
                if 0 <= w < nw:
                    nc.scalar.activation(              # A3: |t| + window sum
                        t2[w % 2][:], t2[w % 2][:], AF.Abs,
                        accum_out=acc[:, w % nwin : w % nwin + 1],
                    ).then_inc(asem, 1)
                else:
                    nc.scalar.activation(scr[:], scr[:], AF.Abs).then_inc(asem, 1)

        @block.gpsimd
        def _(gpsimd):
            for w in range(nw):
                sl = w % nbuf
                x = xy[sl]
                gpsimd.wait_ge(xysem[sl], 16 * (w // nbuf + 1))
                if w >= 1:
                    gpsimd.wait_ge(vsem, 4 * (w - 1) + 3)  # A2(w-2) freed dy
                if c < dfree:
                    nc.gpsimd.tensor_add(                  # J2b
                        dy[w % 2][:, c:dfree],
                        x[:, free + s + c : free + s + dfree],
                        x[:, free + c : free + dfree],
                    ).then_inc(psem, 1)
                else:
                    nc.gpsimd.engine_nop().then_inc(psem, 1)

    return nc


_NC = None


def _get_nc():
    global _NC
    if _NC is None:
        _NC = build_nc()
    return _NC


def shard_host(flat_padded, k, bb=BB, nn=NN, s=S, j=J, nwin=NWIN, pk=PK):
    """[B, N, PTOT] -> core k's [NWIN, B*J, N*S] transposed shard."""
    sl = flat_padded[:, :, k * pk : (k + 1) * pk]          # [B, N, PK]
    v = sl.reshape(bb, nn, nwin, j, s)                     # [B, N, W, J, S]
    v = v.transpose(2, 0, 3, 1, 4)                         # [W, B, J, N, S]
    return np.ascontiguousarray(v).reshape(nwin, bb * j, nn * s)


def _prep_shards(pred, y):
    """Full fp32 inputs -> per-core [NWIN, 128, 2*N*S] fp16 shards with
    odd frames negated (turns frame-diff subtracts into adds)."""
    xf = np.asarray(pred, dtype=np.float32).reshape(BB, NN, HWP)
    yf = np.asarray(y, dtype=np.float32).reshape(BB, NN, HWP)
    sgn = np.where(np.arange(NN) % 2 == 1, -1.0, 1.0).astype(np.float32)
    xf = (xf * sgn[None, :, None]).astype(np.float16)
    yf = (yf * sgn[None, :, None]).astype(np.float16)
    xpad = np.zeros((BB, NN, PTOT), dtype=np.float16)
    ypad = np.zeros((BB, NN, PTOT), dtype=np.float16)
    xpad[:, :, :HWP] = xf
    ypad[:, :, :HWP] = yf
    out = []
    for k in range(NCORES):
        xv = shard_host(xpad, k)
        yv = shard_host(ypad, k)
        out.append({"xy": np.concatenate([xv, yv], axis=2)})
    return out


def _combine(results):
    """Per-core [128, NWIN] window sums -> scalar loss."""
    total = 0.0
    for r in results:
        total += np.asarray(r["partials"], dtype=np.float64).sum()
    return np.array(total / (BB * (NN - 1)), dtype=np.float32)


def run(pred, y, trace=False):
    """Returns (loss, exec_time_ns or None)."""
    nc = _get_nc()
    in_maps = _prep_shards(pred, y)
    res = run_bass_kernel_spmd(
        nc, in_maps, core_ids=list(range(NCORES)), trace=trace
    )
    return _combine(res.results), res.exec_time_ns


def kernel(pred, y):
    out, _ = run(pred, y, trace=False)
    return out
